# revision 20
# baseline (speedup 1.0000x reference)
"""Trainium2 Bass kernel for the CNV binarized CNN (nn_CNV_65549790871624).

Data-parallel over 8 NeuronCores: batch 256 -> 32 images/core. The whole
network runs out of SBUF per core:

  - Input is pre-quantized (8-bit grid, exact in bf16) and im2col'd on host
    so conv1 (C_in=3) is a single K=27 matmul pass.
  - Binary convs run as 3x3 tap-accumulation matmuls into PSUM. Activations
    are +-1 in fp8 stored with a one-pixel zero halo, so every tap is a pure
    strided SBUF read. For the C_in=64 layers (conv2/conv3) a row-shifted
    copy of the activation lives in partitions 64..127, letting vertically
    adjacent tap pairs fuse into single K=128 matmuls (6 passes instead
    of 9).
  - BatchNorm+sign folds to a per-channel threshold compare, executed as one
    ScalarE activation (Sign(x - t)) straight out of PSUM. All conv sums are
    exact integers in f32, so results bit-match the f32 reference up to an
    ~1e-7-wide decision boundary.
  - MaxPool runs on VectorE over the raw conv integers (pooling commutes
    with the monotone threshold), halving ScalarE work.
  - FC layers: FC1 consumes the +-1 activations with a K-permuted weight
    layout chosen so the conv drain writes lhsT tiles directly (no
    transpose). FC2/FC3 consume {0,1} activations (the +-1 correction folds
    into thresholds / a host-side affine), with bf16 DMA-transposes between
    layers. The final TensorNorm affine runs on host in f32, bit-matching
    the reference.
"""

import sys

if "/opt/trn_rl_repo" not in sys.path:
    sys.path.insert(0, "/opt/trn_rl_repo")

from itertools import product

import numpy as np

import concourse.bass as bass
import concourse.mybir as mybir
import concourse.tile as tile
from concourse import bacc
from concourse.bass_utils import run_bass_kernel_spmd

F32N = np.float32
N_CORES = 8
B = 32  # images per core
EPS = 1e-4

FP8 = mybir.dt.float8e4
BF16 = mybir.dt.bfloat16
F32 = mybir.dt.float32
FP8N = mybir.dt.np(FP8)
BF16N = mybir.dt.np(BF16)

TAPS = list(product(range(3), range(3)))  # (ky, kx), tap index t = ky*3+kx
ALU = mybir.AluOpType


# --------------------------------------------------------------------------
# host-side preprocessing
# --------------------------------------------------------------------------

def _thr(bn):
    """Fold BN+sign into a threshold: sign((x-m)*inv+b) == (x >= t), inv>0."""
    g, b, m, v = [np.asarray(a, F32N) for a in bn]
    inv32 = (g / np.sqrt(v + F32N(EPS))).astype(F32N)
    t = m.astype(np.float64) - b.astype(np.float64) / inv32.astype(np.float64)
    return t.astype(F32N)


def _binarize_w(w):
    return np.where(np.asarray(w, F32N) >= 0, F32N(1.0), F32N(-1.0))


def _host_prep(x, conv_ws, bn2d, fc_ws, bn1d):
    x = np.asarray(x, F32N)
    # QuantIdentity: 8-bit, scale 2^-7, bit-matching the jax reference ops
    h = F32N(2.0) * x - F32N(1.0)
    r = np.clip(np.round(h * F32N(128.0)), -128.0, 127.0).astype(F32N)
    q = (r * F32N(2.0**-7)).astype(F32N)  # [256, 3, 32, 32]

    wb = [_binarize_w(w) for w in conv_ws]
    th = [_thr(t) for t in bn2d]

    # L1 im2col: I[3t+c, b, yo, xo] = qpad[c, b, yo+ky, xo+kx]
    qp = np.zeros((3, 256, 34, 34), F32N)
    qp[:, :, 1:33, 1:33] = q.transpose(1, 0, 2, 3)
    I = np.zeros((27, 256, 32, 32), F32N)
    for t, (ky, kx) in enumerate(TAPS):
        I[3 * t : 3 * t + 3] = qp[:, :, ky : ky + 32, kx : kx + 32]
    I = I.astype(BF16N)

    # K is padded to the full 128 partitions everywhere (zero weight rows):
    # sub-128-K matmuls keep the PE HAM-throttled at 1.2 GHz for their whole
    # phase (measured), and zero rows make the extra partitions harmless.
    w1 = np.zeros((128, 64), F32N)
    for t, (ky, kx) in enumerate(TAPS):
        w1[3 * t : 3 * t + 3, :] = wb[0][:, :, ky, kx].T

    def pack_pair_single(w):  # [O, 64, 3, 3] -> wp [128,3,O], ws [128,3,O]
        O = w.shape[0]
        wp = np.zeros((128, 3, O), F32N)
        ws = np.zeros((128, 3, O), F32N)  # rows 64:128 stay zero
        for kx in range(3):
            wp[0:64, kx, :] = w[:, :, 0, kx].T
            wp[64:128, kx, :] = w[:, :, 1, kx].T
            ws[0:64, kx, :] = w[:, :, 2, kx].T
        return wp, ws

    w2p, w2s = pack_pair_single(wb[1])
    w3p, w3s = pack_pair_single(wb[2])

    def pack_full(w):  # [O, C, 3, 3] -> [128, MG, KG, 9, 128]
        O, C = w.shape[0], w.shape[1]
        MG, KG = O // 128, C // 128
        out = np.zeros((128, MG, KG, 9, 128), F32N)
        for mg, kg in product(range(MG), range(KG)):
            for t, (ky, kx) in enumerate(TAPS):
                out[:, mg, kg, t, :] = w[
                    128 * mg : 128 * (mg + 1), 128 * kg : 128 * (kg + 1), ky, kx
                ].T
        return out

    w4 = pack_full(wb[3]).reshape(128, 9, 128)
    w5 = pack_full(wb[4]).reshape(128, 2, 9, 128)
    w6 = pack_full(wb[5])  # [128,2,2,9,128]
    w7 = pack_full(wb[6])  # [128,4,2,9,128]
    w8 = pack_full(wb[7])  # [128,4,4,9,128]

    # negated thresholds packed [128, 16]: L1,L2,L3,L4,L5g0,L5g1,L6g0,L6g1,
    # L7g0..3, L8g0..3
    thc = np.zeros((128, 16), F32N)
    # cols 0/1 replicated into partitions 64:128 for col-tiled psum halves
    thc[0:64, 0] = -th[0]
    thc[64:128, 0] = -th[0]
    thc[0:64, 1] = -th[1]
    thc[64:128, 1] = -th[1]
    thc[:, 2] = -th[2]
    thc[:, 3] = -th[3]
    for mg in range(2):
        thc[:, 4 + mg] = -th[4][128 * mg : 128 * (mg + 1)]
        thc[:, 6 + mg] = -th[5][128 * mg : 128 * (mg + 1)]
    for mg in range(4):
        thc[:, 8 + mg] = -th[6][128 * mg : 128 * (mg + 1)]
        thc[:, 12 + mg] = -th[7][128 * mg : 128 * (mg + 1)]

    # FC prep
    w1b, w2b, w3b = [_binarize_w(w) for w in fc_ws]
    w1t = np.zeros((128, 16, 1024), F32N)
    for g, j in product(range(4), range(4)):
        ks = (128 * g + np.arange(128)) * 4 + j
        w1t[:, 4 * g + j, :] = w1b[:, ks].T
    th_fc1 = _thr(bn1d[0]).reshape(1, 1024)
    t2d = _thr(bn1d[1]).astype(np.float64)
    th_fc2 = ((t2d + w2b.sum(axis=1, dtype=np.float64)) / 2.0).astype(F32N)
    th_fc2 = th_fc2.reshape(1, 512)
    w2t = np.zeros((128, 8, 512), F32N)
    for j in range(8):
        w2t[:, j, :] = w2b[:, 128 * j : 128 * (j + 1)].T
    w3t = np.zeros((128, 4, 200), F32N)
    for j in range(4):
        w3t[:, j, :] = w3b[:, 128 * j : 128 * (j + 1)].T
    rowsum3 = w3b.sum(axis=1, dtype=np.float64).astype(F32N)

    shared = {
        "w1": w1.astype(BF16N),
        "w2p": w2p.astype(FP8N), "w2s": w2s.astype(FP8N),
        "w3p": w3p.astype(FP8N), "w3s": w3s.astype(FP8N),
        "w4": w4.astype(FP8N), "w5": w5.astype(FP8N), "w6": w6.astype(FP8N),
        "w7": w7.astype(FP8N), "w8": w8.astype(FP8N),
        "thc": thc,
        "w1t": w1t.astype(FP8N),
        "w2t": w2t.astype(BF16N), "w3t": w3t.astype(BF16N),
        "thfc1": th_fc1, "thfc2": th_fc2,
    }
    return I, shared, rowsum3


# --------------------------------------------------------------------------
# bass program
# --------------------------------------------------------------------------

def build_program(cache_buster: str | None = None):
    nc = bacc.Bacc("TRN2", target_bir_lowering=False, num_devices=N_CORES)
    if cache_buster:
        # unused dram scratch whose name varies: busts the jax/NEFF compile
        # cache when we want to A/B walrus flags on an identical program
        nc.dram_tensor(f"cachebust_{cache_buster}", [1, 1], F32)

    def par(name, shape, dt):
        return nc.declare_dram_parameter(name, shape, dt, isOutput=False)

    p_im = par("im2col", [27, B, 32, 32], BF16)
    p_w1 = par("w1", [128, 64], BF16)
    p_w2p = par("w2p", [128, 3, 64], FP8)
    p_w2s = par("w2s", [128, 3, 64], FP8)
    p_w3p = par("w3p", [128, 3, 128], FP8)
    p_w3s = par("w3s", [128, 3, 128], FP8)
    p_w4 = par("w4", [128, 9, 128], FP8)
    p_w5 = par("w5", [128, 2, 9, 128], FP8)
    p_w6 = par("w6", [128, 2, 2, 9, 128], FP8)
    p_w7 = par("w7", [128, 4, 2, 9, 128], FP8)
    p_w8 = par("w8", [128, 4, 4, 9, 128], FP8)
    p_thc = par("thc", [128, 16], F32)
    p_w1t = par("w1t", [128, 16, 1024], FP8)
    p_w2t = par("w2t", [128, 8, 512], BF16)
    p_w3t = par("w3t", [128, 4, 200], BF16)
    p_thfc1 = par("thfc1", [1, 1024], F32)
    p_thfc2 = par("thfc2", [1, 512], F32)
    p_out = nc.declare_dram_parameter("out", [B, 200], F32, isOutput=True)

    with tile.TileContext(nc) as tc:
        with (
            tc.tile_pool(name="consts", bufs=1) as consts,
            tc.tile_pool(name="acts", bufs=1) as acts,
            tc.tile_pool(name="imp", bufs=2) as imp,
            tc.tile_pool(name="tmp", bufs=4) as tmp,
            tc.tile_pool(name="psum", bufs=6, space="PSUM") as psum,
        ):
            # ---- load constants ----
            def cload(p, shape, dt, tag):
                t = consts.tile(shape, dt, tag=tag)
                nc.sync.dma_start(out=t, in_=p[(slice(None),) * len(shape)])
                return t

            # only what L1 needs up front; heavier weight loads are emitted
            # between layers so they queue behind each layer's small DMAs and
            # stream in during compute instead of blocking the first im2col
            thc_s = cload(p_thc, [128, 16], F32, "thc")
            w1_s = cload(p_w1, [128, 64], BF16, "w1")
            w2p_s = cload(p_w2p, [128, 3, 64], FP8, "w2p")
            w2s_s = cload(p_w2s, [128, 3, 64], FP8, "w2s")

            def th(col, parts=128):
                return thc_s[0:parts, col : col + 1]

            # ---- activation buffers (one-pixel zero halo) ----
            AB1 = acts.tile([128, B, 34, 34], FP8, tag="AB1")
            AB2 = acts.tile([128, B, 18, 18], FP8, tag="AB2")
            A3 = acts.tile([128, B, 18, 18], FP8, tag="A3")
            A4 = acts.tile([128, B, 10, 10], FP8, tag="A4")
            A5 = acts.tile([128, 2, B, 10, 10], FP8, tag="A5")
            A6 = acts.tile([128, 2, B, 6, 6], FP8, tag="A6")
            A7 = acts.tile([128, 4, B, 6, 6], FP8, tag="A7")
            HT = acts.tile([128, 16, B], FP8, tag="HT")
            H2 = acts.tile([B, 1024], BF16, tag="H2")
            H2T = acts.tile([128, 8, B], BF16, tag="H2T")
            H3 = acts.tile([B, 512], BF16, tag="H3")
            H3T = acts.tile([128, 4, B], BF16, tag="H3T")
            OUTS = acts.tile([B, 200], F32, tag="OUTS")

            def zero_halo2(buf, parts, S):  # buf [128, B, S+2, S+2]
                nc.gpsimd.memset(buf[0:parts, :, 0, :], 0.0)
                nc.gpsimd.memset(buf[0:parts, :, S + 1, :], 0.0)
                nc.gpsimd.memset(buf[0:parts, :, :, 0], 0.0)
                nc.gpsimd.memset(buf[0:parts, :, :, S + 1], 0.0)

            def zero_halo3(buf, S):  # buf [128, G, B, S+2, S+2]
                nc.gpsimd.memset(buf[:, :, :, 0, :], 0.0)
                nc.gpsimd.memset(buf[:, :, :, S + 1, :], 0.0)
                nc.gpsimd.memset(buf[:, :, :, :, 0], 0.0)
                nc.gpsimd.memset(buf[:, :, :, :, S + 1], 0.0)

            zero_halo2(AB1, 64, 32)
            zero_halo2(AB2, 64, 16)
            # AB1 B-half is now scattered directly (interior cols 1:33, rows
            # 0:32): zero its halo cols and bottom rows (rows 32/33 are only
            # read zero-weighted; cols 0/33 are real halo for the pairs)
            nc.gpsimd.memset(AB1[64:128, :, :, 0], 0.0)
            nc.gpsimd.memset(AB1[64:128, :, :, 33], 0.0)
            nc.gpsimd.memset(AB1[64:128, :, 32, 1:33], 0.0)
            nc.gpsimd.memset(AB1[64:128, :, 33, 1:33], 0.0)
            nc.gpsimd.memset(AB2[64:128, :, 17, :], 0.0)
            zero_halo2(A3, 128, 16)
            zero_halo2(A4, 128, 8)
            zero_halo3(A5, 8)
            zero_halo3(A6, 4)
            zero_halo3(A7, 4)

            # ---- L1: im2col matmul, K=27 padded to 128 ----
            for c4 in range(8):  # 4-image input chunks
                im_s = imp.tile([128, 4, 32, 32], BF16, tag="imc")
                # zero whole tile first: the DMA only fills 0:27, and the
                # zero-weight K-padding rows must still multiply finite data
                nc.gpsimd.memset(im_s, 0.0)
                nc.sync.dma_start(
                    out=im_s[0:27], in_=p_im[:, 4 * c4 : 4 * c4 + 4, :, :]
                )
                for bi in range(4):
                    b = 4 * c4 + bi
                    # 2-way col tiling: y-half 0 -> psum parts 0:64 (array
                    # cols 0:64), y-half 1 -> parts 64:128 (cols 64:128)
                    ps = psum.tile([128, 16, 32], F32, tag="ps")
                    nc.tensor.matmul(
                        ps[0:64], w1_s, im_s[:, bi, 0:16, :],
                        start=True, stop=True, tile_position=(0, 0),
                    )
                    nc.tensor.matmul(
                        ps[64:128], w1_s, im_s[:, bi, 16:32, :],
                        start=True, stop=True, tile_position=(0, 64),
                    )
                    s1 = tmp.tile([128, 16, 32], FP8, tag="s1")
                    nc.scalar.sign(s1, ps, bias=thc_s[:, 0:1])
                    # scatter the fp8 signs into the padded A-half interior
                    # and the row-shifted B-half (B[y] = A[y+1]) via DMA
                    nc.sync.dma_start(
                        out=AB1[0:64, b, 1:17, 1:33], in_=s1[0:64]
                    )
                    nc.sync.dma_start(
                        out=AB1[0:64, b, 17:33, 1:33], in_=s1[64:128]
                    )
                    nc.sync.dma_start(
                        out=AB1[64:128, b, 0:16, 1:33], in_=s1[0:64]
                    )
                    nc.sync.dma_start(
                        out=AB1[64:128, b, 16:32, 1:33], in_=s1[64:128]
                    )

            w3p_s = cload(p_w3p, [128, 3, 128], FP8, "w3p")
            w3s_s = cload(p_w3s, [128, 3, 128], FP8, "w3s")
            w4_s = cload(p_w4, [128, 9, 128], FP8, "w4")

            # ---- L2 (64->64) + pool, 2-way col tiling over y-halves.
            # Each half gets its own PSUM bank: start=True clears has_written
            # for the whole bank, so interleaved accumulation of both halves
            # must not share one.
            for b in range(B):
                psA = psum.tile([128, 16, 32], F32, tag="ps")
                psB = psum.tile([128, 16, 32], F32, tag="ps")
                ph = [psA[0:64], psB[64:128]]
                y0s = [0, 16]
                for kx in range(3):  # ky=0,1 pairs (K=128)
                    for h in range(2):
                        nc.tensor.matmul(
                            ph[h], w2p_s[:, kx, :],
                            AB1[:, b, y0s[h] : y0s[h] + 16, kx : kx + 32],
                            start=(kx == 0), stop=False,
                            tile_position=(0, 64 * h),
                        )
                for kx in range(3):  # ky=2 singles (K padded to 128)
                    for h in range(2):
                        nc.tensor.matmul(
                            ph[h], w2s_s[:, kx, :],
                            AB1[:, b, y0s[h] + 2 : y0s[h] + 18, kx : kx + 32],
                            start=False, stop=(kx == 2),
                            tile_position=(0, 64 * h),
                        )
                for h in range(2):
                    pslc = ph[h]
                    psv = pslc.rearrange("c h (w two) -> c h w two", two=2)
                    te = tmp.tile([64, 16, 16], F32, tag="te")
                    nc.vector.tensor_copy(te, psv[:, :, :, 0])
                    t1 = tmp.tile([64, 16, 16], F32, tag="t1")
                    nc.vector.tensor_max(t1, psv[:, :, :, 1], te)
                    t1v = t1.rearrange("c (h two) w -> c h two w", two=2)
                    t2 = tmp.tile([64, 8, 16], F32, tag="t2")
                    nc.vector.tensor_max(t2, t1v[:, :, 0, :], t1v[:, :, 1, :])
                    yp0 = 8 * h
                    nc.scalar.sign(
                        AB2[0:64, b, 1 + yp0 : 9 + yp0, 1:17], t2,
                        bias=thc_s[0:64, 1:2],
                    )
                nc.sync.dma_start(
                    out=AB2[64:128, b, 0:17, :], in_=AB2[0:64, b, 1:18, :]
                )

            w5_s = cload(p_w5, [128, 2, 9, 128], FP8, "w5")
            w6_s = cload(p_w6, [128, 2, 2, 9, 128], FP8, "w6")

            # ---- L3 (64->128) ----
            for ci in range(16):  # 2 images per chunk
                b0 = 2 * ci
                ps = psum.tile([128, 2, 16, 16], F32, tag="ps")
                for kx in range(3):
                    nc.tensor.matmul(
                        ps, w3p_s[:, kx, :],
                        AB2[:, b0 : b0 + 2, 0:16, kx : kx + 16],
                        start=(kx == 0), stop=False,
                    )
                for kx in range(3):
                    nc.tensor.matmul(
                        ps, w3s_s[:, kx, :],
                        AB2[:, b0 : b0 + 2, 2:18, kx : kx + 16],
                        start=False, stop=(kx == 2),
                    )
                nc.scalar.sign(A3[:, b0 : b0 + 2, 1:17, 1:17], ps, bias=th(2))

            w7_s = cload(p_w7, [128, 4, 2, 9, 128], FP8, "w7")

            # ---- L4 (128->128) + pool ----
            for ci in range(16):
                b0 = 2 * ci
                ps = psum.tile([128, 2, 16, 16], F32, tag="ps")
                for t, (ky, kx) in enumerate(TAPS):
                    nc.tensor.matmul(
                        ps, w4_s[:, t, :],
                        A3[:, b0 : b0 + 2, ky : ky + 16, kx : kx + 16],
                        start=(t == 0), stop=(t == 8),
                    )
                psv = ps.rearrange("c b h (w two) -> c b h w two", two=2)
                te = tmp.tile([128, 2, 16, 8], F32, tag="te")
                nc.vector.tensor_copy(te, psv[:, :, :, :, 0])
                t1 = tmp.tile([128, 2, 16, 8], F32, tag="t1")
                nc.vector.tensor_max(t1, psv[:, :, :, :, 1], te)
                t1v = t1.rearrange("c b (h two) w -> c b h two w", two=2)
                t2 = tmp.tile([128, 2, 8, 8], F32, tag="t2")
                nc.vector.tensor_max(t2, t1v[:, :, :, 0, :], t1v[:, :, :, 1, :])
                nc.scalar.sign(A4[:, b0 : b0 + 2, 1:9, 1:9], t2, bias=th(3))

            w8_s = cload(p_w8, [128, 4, 4, 9, 128], FP8, "w8")

            # ---- L5 (128->256) ----
            for ci in range(4):  # 8 images per chunk
                b0 = 8 * ci
                for mg in range(2):
                    ps = psum.tile([128, 8, 8, 8], F32, tag="ps")
                    for t, (ky, kx) in enumerate(TAPS):
                        nc.tensor.matmul(
                            ps, w5_s[:, mg, t, :],
                            A4[:, b0 : b0 + 8, ky : ky + 8, kx : kx + 8],
                            start=(t == 0), stop=(t == 8),
                        )
                    nc.scalar.sign(
                        A5[:, mg, b0 : b0 + 8, 1:9, 1:9], ps, bias=th(4 + mg)
                    )

            w1t_s = cload(p_w1t, [128, 16, 1024], FP8, "w1t")
            thfc1_s = consts.tile([B, 1024], F32, tag="thfc1")
            nc.sync.dma_start(out=thfc1_s, in_=p_thfc1[:, :].to_broadcast((B, 1024)))

            # ---- L6 (256->256) + pool ----
            for ci in range(4):
                b0 = 8 * ci
                for mg in range(2):
                    ps = psum.tile([128, 8, 8, 8], F32, tag="ps")
                    i = 0
                    for kg in range(2):
                        for t, (ky, kx) in enumerate(TAPS):
                            nc.tensor.matmul(
                                ps, w6_s[:, mg, kg, t, :],
                                A5[:, kg, b0 : b0 + 8, ky : ky + 8, kx : kx + 8],
                                start=(i == 0), stop=(i == 17),
                            )
                            i += 1
                    psv = ps.rearrange("c b h (w two) -> c b h w two", two=2)
                    te = tmp.tile([128, 8, 8, 4], F32, tag="te")
                    nc.vector.tensor_copy(te, psv[:, :, :, :, 0])
                    t1 = tmp.tile([128, 8, 8, 4], F32, tag="t1")
                    nc.vector.tensor_max(t1, psv[:, :, :, :, 1], te)
                    t1v = t1.rearrange("c b (h two) w -> c b h two w", two=2)
                    t2 = tmp.tile([128, 8, 4, 4], F32, tag="t2")
                    nc.vector.tensor_max(
                        t2, t1v[:, :, :, 0, :], t1v[:, :, :, 1, :]
                    )
                    nc.scalar.sign(
                        A6[:, mg, b0 : b0 + 8, 1:5, 1:5], t2, bias=th(6 + mg)
                    )

            w2t_s = cload(p_w2t, [128, 8, 512], BF16, "w2t")
            w3t_s = cload(p_w3t, [128, 4, 200], BF16, "w3t")
            thfc2_s = consts.tile([B, 512], F32, tag="thfc2")
            nc.sync.dma_start(out=thfc2_s, in_=p_thfc2[:, :].to_broadcast((B, 512)))

            # ---- L7 (256->512) ----
            for mg in range(4):
                ps = psum.tile([128, B, 4, 4], F32, tag="ps")
                i = 0
                for kg in range(2):
                    for t, (ky, kx) in enumerate(TAPS):
                        nc.tensor.matmul(
                            ps, w7_s[:, mg, kg, t, :],
                            A6[:, kg, :, ky : ky + 4, kx : kx + 4],
                            start=(i == 0), stop=(i == 17),
                        )
                        i += 1
                nc.scalar.sign(A7[:, mg, :, 1:5, 1:5], ps, bias=th(8 + mg))

            # ---- L8 (512->512) + pool -> HT ----
            for mg in range(4):
                ps = psum.tile([128, B, 4, 4], F32, tag="ps")
                i = 0
                for kg in range(4):
                    for t, (ky, kx) in enumerate(TAPS):
                        nc.tensor.matmul(
                            ps, w8_s[:, mg, kg, t, :],
                            A7[:, kg, :, ky : ky + 4, kx : kx + 4],
                            start=(i == 0), stop=(i == 35),
                        )
                        i += 1
                psv = ps.rearrange("c b h (w two) -> c b h w two", two=2)
                te = tmp.tile([128, B, 4, 2], F32, tag="te")
                nc.vector.tensor_copy(te, psv[:, :, :, :, 0])
                t1 = tmp.tile([128, B, 4, 2], F32, tag="t1")
                nc.vector.tensor_max(t1, psv[:, :, :, :, 1], te)
                t1v = t1.rearrange("c b (h two) w -> c b h two w", two=2)
                t2 = tmp.tile([128, B, 2, 2], F32, tag="t2")
                nc.vector.tensor_max(t2, t1v[:, :, :, 0, :], t1v[:, :, :, 1, :])
                for s in range(4):
                    nc.scalar.sign(
                        HT[:, 4 * mg + s, :], t2[:, :, s // 2, s % 2],
                        bias=th(12 + mg),
                    )

            # ---- FC1 (2048->1024), +-1 inputs ----
            psa = psum.tile([B, 512], F32, tag="ps")
            psb = psum.tile([B, 512], F32, tag="ps")
            DRM = mybir.MatmulPerfMode.DoubleRow
            for t in range(8):  # DoubleRow: K=256 per pass (fp8)
                nc.tensor.matmul(
                    psa, HT[:, 2 * t : 2 * t + 2, :],
                    w1t_s[:, 2 * t : 2 * t + 2, 0:512],
                    start=(t == 0), stop=(t == 7), perf_mode=DRM,
                )
            for t in range(8):
                nc.tensor.matmul(
                    psb, HT[:, 2 * t : 2 * t + 2, :],
                    w1t_s[:, 2 * t : 2 * t + 2, 512:1024],
                    start=(t == 0), stop=(t == 7), perf_mode=DRM,
                )
            nc.vector.tensor_tensor(
                H2[:, 0:512], psa, thfc1_s[:, 0:512], op=ALU.is_ge
            )
            nc.vector.tensor_tensor(
                H2[:, 512:1024], psb, thfc1_s[:, 512:1024], op=ALU.is_ge
            )
            for j in range(8):
                nc.sync.dma_start(
                    out=H2T[:, j, :], in_=H2[:, 128 * j : 128 * (j + 1)],
                    transpose=True,
                )

            # ---- FC2 (1024->512), {0,1} inputs ----
            ps2 = psum.tile([B, 512], F32, tag="ps")
            for j in range(8):
                nc.tensor.matmul(
                    ps2, H2T[:, j, :], w2t_s[:, j, :],
                    start=(j == 0), stop=(j == 7),
                )
            nc.vector.tensor_tensor(H3, ps2, thfc2_s, op=ALU.is_ge)
            for j in range(4):
                nc.sync.dma_start(
                    out=H3T[:, j, :], in_=H3[:, 128 * j : 128 * (j + 1)],
                    transpose=True,
                )

            # ---- FC3 (512->200), raw accumulator out ----
            ps3 = psum.tile([B, 200], F32, tag="ps")
            for j in range(4):
                nc.tensor.matmul(
                    ps3, H3T[:, j, :], w3t_s[:, j, :],
                    start=(j == 0), stop=(j == 3),
                )
            nc.scalar.copy(OUTS, ps3)
            nc.sync.dma_start(out=p_out[:, :], in_=OUTS)

    nc.finalize()
    return nc


_PROGRAM = None
TRACE = False  # set True (e.g. from test.py) to capture an NTFF profile
LAST_RESULTS = None


def _get_program():
    global _PROGRAM
    if _PROGRAM is None:
        _PROGRAM = build_program()
    return _PROGRAM


# --------------------------------------------------------------------------
# entry point
# --------------------------------------------------------------------------

def kernel(x, conv_ws, bn2d, fc_ws, bn1d, tn):
    I, shared, rowsum3 = _host_prep(x, conv_ws, bn2d, fc_ws, bn1d)

    nc = _get_program()
    in_maps = []
    for c in range(N_CORES):
        m = dict(shared)
        m["im2col"] = np.ascontiguousarray(I[:, B * c : B * (c + 1)])
        in_maps.append(m)

    global LAST_RESULTS
    res = run_bass_kernel_spmd(nc, in_maps, list(range(N_CORES)), trace=TRACE)
    LAST_RESULTS = res
    fc3 = np.concatenate(
        [res.results[c]["out"] for c in range(N_CORES)], axis=0
    ).astype(F32N)  # [256, 200], FC3 sums over {0,1} inputs

    # undo the {0,1} encoding, then TensorNorm (all f32, matching reference)
    true3 = F32N(2.0) * fc3 - rowsum3[None, :]
    tw, tb, tm, tv = [F32N(np.asarray(a)) for a in tn]
    out = (true3 - tm) / np.sqrt(tv + F32N(EPS)) * tw + tb
    return out.astype(F32N)


# revision 21
# speedup vs baseline: 1.8500x; 1.8500x over previous
"""Trainium2 Bass kernel for the CNV binarized CNN (nn_CNV_65549790871624).

Data-parallel over 8 NeuronCores: batch 256 -> 32 images/core. The whole
network runs out of SBUF per core:

  - Input is pre-quantized (8-bit grid, exact in bf16) and im2col'd on host
    so conv1 (C_in=3) is a single K=27 matmul pass.
  - Binary convs run as 3x3 tap-accumulation matmuls into PSUM. Activations
    are +-1 in fp8 stored with a one-pixel zero halo, so every tap is a pure
    strided SBUF read. For the C_in=64 layers (conv2/conv3) a row-shifted
    copy of the activation lives in partitions 64..127, letting vertically
    adjacent tap pairs fuse into single K=128 matmuls (6 passes instead
    of 9).
  - BatchNorm+sign folds to a per-channel threshold compare, executed as one
    ScalarE activation (Sign(x - t)) straight out of PSUM. All conv sums are
    exact integers in f32, so results bit-match the f32 reference up to an
    ~1e-7-wide decision boundary.
  - MaxPool runs on VectorE over the raw conv integers (pooling commutes
    with the monotone threshold), halving ScalarE work.
  - FC layers: FC1 consumes the +-1 activations with a K-permuted weight
    layout chosen so the conv drain writes lhsT tiles directly (no
    transpose). FC2/FC3 consume {0,1} activations (the +-1 correction folds
    into thresholds / a host-side affine), with bf16 DMA-transposes between
    layers. The final TensorNorm affine runs on host in f32, bit-matching
    the reference.
"""

import sys

if "/opt/trn_rl_repo" not in sys.path:
    sys.path.insert(0, "/opt/trn_rl_repo")

from itertools import product

import numpy as np

import concourse.bass as bass
import concourse.mybir as mybir
import concourse.tile as tile
from concourse import bacc
from concourse.bass_utils import run_bass_kernel_spmd

F32N = np.float32
N_CORES = 8
B = 32  # images per core
EPS = 1e-4

FP8 = mybir.dt.float8e4
BF16 = mybir.dt.bfloat16
F32 = mybir.dt.float32
FP8N = mybir.dt.np(FP8)
BF16N = mybir.dt.np(BF16)

TAPS = list(product(range(3), range(3)))  # (ky, kx), tap index t = ky*3+kx
ALU = mybir.AluOpType


# --------------------------------------------------------------------------
# host-side preprocessing
# --------------------------------------------------------------------------

def _thr(bn):
    """Fold BN+sign into a threshold: sign((x-m)*inv+b) == (x >= t), inv>0."""
    g, b, m, v = [np.asarray(a, F32N) for a in bn]
    inv32 = (g / np.sqrt(v + F32N(EPS))).astype(F32N)
    t = m.astype(np.float64) - b.astype(np.float64) / inv32.astype(np.float64)
    return t.astype(F32N)


def _binarize_w(w):
    return np.where(np.asarray(w, F32N) >= 0, F32N(1.0), F32N(-1.0))


def _host_prep(x, conv_ws, bn2d, fc_ws, bn1d):
    x = np.asarray(x, F32N)
    # QuantIdentity: 8-bit, scale 2^-7, bit-matching the jax reference ops
    h = F32N(2.0) * x - F32N(1.0)
    r = np.clip(np.round(h * F32N(128.0)), -128.0, 127.0).astype(F32N)
    q = (r * F32N(2.0**-7)).astype(F32N)  # [256, 3, 32, 32]

    wb = [_binarize_w(w) for w in conv_ws]
    th = [_thr(t) for t in bn2d]

    # L1 im2col: I[3t+c, b, yo, xo] = qpad[c, b, yo+ky, xo+kx]
    qp = np.zeros((3, 256, 34, 34), F32N)
    qp[:, :, 1:33, 1:33] = q.transpose(1, 0, 2, 3)
    I = np.zeros((27, 256, 32, 32), F32N)
    for t, (ky, kx) in enumerate(TAPS):
        I[3 * t : 3 * t + 3] = qp[:, :, ky : ky + 32, kx : kx + 32]
    I = I.astype(BF16N)

    # K is padded to the full 128 partitions everywhere (zero weight rows):
    # sub-128-K matmuls keep the PE HAM-throttled at 1.2 GHz for their whole
    # phase (measured), and zero rows make the extra partitions harmless.
    w1 = np.zeros((27, 64), F32N)
    for t, (ky, kx) in enumerate(TAPS):
        w1[3 * t : 3 * t + 3, :] = wb[0][:, :, ky, kx].T

    def pack_pair_single(w):  # [O, 64, 3, 3] -> wp [128,3,O], ws [128,3,O]
        O = w.shape[0]
        wp = np.zeros((128, 3, O), F32N)
        ws = np.zeros((128, 3, O), F32N)  # rows 64:128 stay zero
        for kx in range(3):
            wp[0:64, kx, :] = w[:, :, 0, kx].T
            wp[64:128, kx, :] = w[:, :, 1, kx].T
            ws[0:64, kx, :] = w[:, :, 2, kx].T
        return wp, ws

    w2p, w2s = pack_pair_single(wb[1])
    w3p, w3s = pack_pair_single(wb[2])

    def pack_full(w):  # [O, C, 3, 3] -> [128, MG, KG, 9, 128]
        O, C = w.shape[0], w.shape[1]
        MG, KG = O // 128, C // 128
        out = np.zeros((128, MG, KG, 9, 128), F32N)
        for mg, kg in product(range(MG), range(KG)):
            for t, (ky, kx) in enumerate(TAPS):
                out[:, mg, kg, t, :] = w[
                    128 * mg : 128 * (mg + 1), 128 * kg : 128 * (kg + 1), ky, kx
                ].T
        return out

    w4 = pack_full(wb[3]).reshape(128, 9, 128)
    w5 = pack_full(wb[4]).reshape(128, 2, 9, 128)
    w6 = pack_full(wb[5])  # [128,2,2,9,128]
    w7 = pack_full(wb[6])  # [128,4,2,9,128]
    w8 = pack_full(wb[7])  # [128,4,4,9,128]

    # negated thresholds packed [128, 16]: L1,L2,L3,L4,L5g0,L5g1,L6g0,L6g1,
    # L7g0..3, L8g0..3
    thc = np.zeros((128, 16), F32N)
    # cols 0/1 replicated into partitions 64:128 for col-tiled psum halves
    thc[0:64, 0] = -th[0]
    thc[64:128, 0] = -th[0]
    thc[0:64, 1] = -th[1]
    thc[64:128, 1] = -th[1]
    thc[:, 2] = -th[2]
    thc[:, 3] = -th[3]
    for mg in range(2):
        thc[:, 4 + mg] = -th[4][128 * mg : 128 * (mg + 1)]
        thc[:, 6 + mg] = -th[5][128 * mg : 128 * (mg + 1)]
    for mg in range(4):
        thc[:, 8 + mg] = -th[6][128 * mg : 128 * (mg + 1)]
        thc[:, 12 + mg] = -th[7][128 * mg : 128 * (mg + 1)]

    # FC prep
    w1b, w2b, w3b = [_binarize_w(w) for w in fc_ws]
    w1t = np.zeros((128, 16, 1024), F32N)
    for g, j in product(range(4), range(4)):
        ks = (128 * g + np.arange(128)) * 4 + j
        w1t[:, 4 * g + j, :] = w1b[:, ks].T
    th_fc1 = _thr(bn1d[0]).reshape(1, 1024)
    t2d = _thr(bn1d[1]).astype(np.float64)
    th_fc2 = ((t2d + w2b.sum(axis=1, dtype=np.float64)) / 2.0).astype(F32N)
    th_fc2 = th_fc2.reshape(1, 512)
    w2t = np.zeros((128, 8, 512), F32N)
    for j in range(8):
        w2t[:, j, :] = w2b[:, 128 * j : 128 * (j + 1)].T
    w3t = np.zeros((128, 4, 200), F32N)
    for j in range(4):
        w3t[:, j, :] = w3b[:, 128 * j : 128 * (j + 1)].T
    rowsum3 = w3b.sum(axis=1, dtype=np.float64).astype(F32N)

    shared = {
        "w1": w1.astype(BF16N),
        "w2p": w2p.astype(FP8N), "w2s": w2s.astype(FP8N),
        "w3p": w3p.astype(FP8N), "w3s": w3s.astype(FP8N),
        "w4": w4.astype(FP8N), "w5": w5.astype(FP8N), "w6": w6.astype(FP8N),
        "w7": w7.astype(FP8N), "w8": w8.astype(FP8N),
        "thc": thc,
        "w1t": w1t.astype(FP8N),
        "w2t": w2t.astype(BF16N), "w3t": w3t.astype(BF16N),
        "thfc1": th_fc1, "thfc2": th_fc2,
    }
    return I, shared, rowsum3


# --------------------------------------------------------------------------
# bass program
# --------------------------------------------------------------------------

def build_program(cache_buster: str | None = None):
    nc = bacc.Bacc("TRN2", target_bir_lowering=False, num_devices=N_CORES)
    if cache_buster:
        # unused dram scratch whose name varies: busts the jax/NEFF compile
        # cache when we want to A/B walrus flags on an identical program
        nc.dram_tensor(f"cachebust_{cache_buster}", [1, 1], F32)

    def par(name, shape, dt):
        return nc.declare_dram_parameter(name, shape, dt, isOutput=False)

    p_im = par("im2col", [27, B, 32, 32], BF16)
    p_w1 = par("w1", [27, 64], BF16)
    p_w2p = par("w2p", [128, 3, 64], FP8)
    p_w2s = par("w2s", [128, 3, 64], FP8)
    p_w3p = par("w3p", [128, 3, 128], FP8)
    p_w3s = par("w3s", [128, 3, 128], FP8)
    p_w4 = par("w4", [128, 9, 128], FP8)
    p_w5 = par("w5", [128, 2, 9, 128], FP8)
    p_w6 = par("w6", [128, 2, 2, 9, 128], FP8)
    p_w7 = par("w7", [128, 4, 2, 9, 128], FP8)
    p_w8 = par("w8", [128, 4, 4, 9, 128], FP8)
    p_thc = par("thc", [128, 16], F32)
    p_w1t = par("w1t", [128, 16, 1024], FP8)
    p_w2t = par("w2t", [128, 8, 512], BF16)
    p_w3t = par("w3t", [128, 4, 200], BF16)
    p_thfc1 = par("thfc1", [1, 1024], F32)
    p_thfc2 = par("thfc2", [1, 512], F32)
    p_out = nc.declare_dram_parameter("out", [B, 200], F32, isOutput=True)

    with tile.TileContext(nc) as tc:
        with (
            tc.tile_pool(name="consts", bufs=1) as consts,
            tc.tile_pool(name="acts", bufs=1) as acts,
            tc.tile_pool(name="imp", bufs=2) as imp,
            tc.tile_pool(name="tmp", bufs=4) as tmp,
            tc.tile_pool(name="psum", bufs=8, space="PSUM") as psum,
        ):
            # ---- load constants ----
            def cload(p, shape, dt, tag):
                t = consts.tile(shape, dt, tag=tag)
                nc.sync.dma_start(out=t, in_=p[(slice(None),) * len(shape)])
                return t

            # only what L1 needs up front; heavier weight loads are emitted
            # between layers so they queue behind each layer's small DMAs and
            # stream in during compute instead of blocking the first im2col
            thc_s = cload(p_thc, [128, 16], F32, "thc")
            w1_s = cload(p_w1, [27, 64], BF16, "w1")
            w2p_s = cload(p_w2p, [128, 3, 64], FP8, "w2p")
            w2s_s = cload(p_w2s, [128, 3, 64], FP8, "w2s")

            def th(col, parts=128):
                return thc_s[0:parts, col : col + 1]

            # ---- activation buffers (one-pixel zero halo) ----
            AB1 = acts.tile([128, B, 34, 34], FP8, tag="AB1")
            AB2 = acts.tile([128, B, 18, 18], FP8, tag="AB2")
            A3 = acts.tile([128, B, 18, 18], FP8, tag="A3")
            A4 = acts.tile([128, B, 10, 10], FP8, tag="A4")
            A5 = acts.tile([128, 2, B, 10, 10], FP8, tag="A5")
            A6 = acts.tile([128, 2, B, 6, 6], FP8, tag="A6")
            A7 = acts.tile([128, 4, B, 6, 6], FP8, tag="A7")
            HT = acts.tile([128, 16, B], FP8, tag="HT")
            H2 = acts.tile([B, 1024], BF16, tag="H2")
            H2T = acts.tile([128, 8, B], BF16, tag="H2T")
            H3 = acts.tile([B, 512], BF16, tag="H3")
            H3T = acts.tile([128, 4, B], BF16, tag="H3T")
            OUTS = acts.tile([B, 200], F32, tag="OUTS")

            def zero_halo2(buf, parts, S):  # buf [128, B, S+2, S+2]
                nc.vector.memset(buf[0:parts, :, 0, :], 0.0)
                nc.vector.memset(buf[0:parts, :, S + 1, :], 0.0)
                nc.gpsimd.memset(buf[0:parts, :, :, 0], 0.0)
                nc.gpsimd.memset(buf[0:parts, :, :, S + 1], 0.0)

            def zero_halo3(buf, S):  # buf [128, G, B, S+2, S+2]
                nc.vector.memset(buf[:, :, :, 0, :], 0.0)
                nc.vector.memset(buf[:, :, :, S + 1, :], 0.0)
                nc.gpsimd.memset(buf[:, :, :, :, 0], 0.0)
                nc.gpsimd.memset(buf[:, :, :, :, S + 1], 0.0)

            zero_halo2(AB1, 64, 32)
            zero_halo2(AB2, 64, 16)
            # B-half bottom rows are never written by the shift-copy but are
            # read (x0-weighted) by the K-padded "single" matmuls
            nc.vector.memset(AB1[64:128, :, 33, :], 0.0)
            nc.vector.memset(AB2[64:128, :, 17, :], 0.0)
            zero_halo2(A3, 128, 16)
            zero_halo2(A4, 128, 8)
            zero_halo3(A5, 8)
            zero_halo3(A6, 4)
            zero_halo3(A7, 4)

            # ---- L1: im2col matmul, K=27 padded to 128 ----
            for c4 in range(8):  # 4-image input chunks
                im_s = imp.tile([27, 4, 32, 32], BF16, tag="imc")
                nc.sync.dma_start(
                    out=im_s, in_=p_im[:, 4 * c4 : 4 * c4 + 4, :, :]
                )
                for bi in range(4):
                    b = 4 * c4 + bi
                    # 2-way col tiling: y-half 0 -> psum parts 0:64 (array
                    # cols 0:64), y-half 1 -> parts 64:128 (cols 64:128)
                    ps = psum.tile([128, 16, 32], F32, tag="ps")
                    nc.tensor.matmul(
                        ps[0:64], w1_s, im_s[:, bi, 0:16, :],
                        start=True, stop=True, tile_position=(0, 0),
                    )
                    nc.tensor.matmul(
                        ps[64:128], w1_s, im_s[:, bi, 16:32, :],
                        start=True, stop=True, tile_position=(0, 64),
                    )
                    nc.scalar.sign(
                        AB1[0:64, b, 1:17, 1:33], ps[0:64], bias=thc_s[0:64, 0:1]
                    )
                    nc.scalar.sign(
                        AB1[0:64, b, 17:33, 1:33], ps[64:128],
                        bias=thc_s[64:128, 0:1],
                    )
                    # B-half: shift one row up (B[y] = A[y+1])
                    nc.sync.dma_start(
                        out=AB1[64:128, b, 0:33, :], in_=AB1[0:64, b, 1:34, :]
                    )

            w3p_s = cload(p_w3p, [128, 3, 128], FP8, "w3p")
            w3s_s = cload(p_w3s, [128, 3, 128], FP8, "w3s")
            w4_s = cload(p_w4, [128, 9, 128], FP8, "w4")

            # ---- L2 (64->64) + pool, 2-way col tiling over y-halves.
            # Each half gets its own PSUM bank: start=True clears has_written
            # for the whole bank, so interleaved accumulation of both halves
            # must not share one.
            for b in range(B):
                psA = psum.tile([128, 16, 32], F32, tag="ps")
                psB = psum.tile([128, 16, 32], F32, tag="ps")
                ph = [psA[0:64], psB[64:128]]
                y0s = [0, 16]
                for kx in range(3):  # ky=0,1 pairs (K=128)
                    for h in range(2):
                        nc.tensor.matmul(
                            ph[h], w2p_s[:, kx, :],
                            AB1[:, b, y0s[h] : y0s[h] + 16, kx : kx + 32],
                            start=(kx == 0), stop=False,
                            tile_position=(0, 64 * h),
                        )
                for kx in range(3):  # ky=2 singles (K padded to 128)
                    for h in range(2):
                        nc.tensor.matmul(
                            ph[h], w2s_s[:, kx, :],
                            AB1[:, b, y0s[h] + 2 : y0s[h] + 18, kx : kx + 32],
                            start=False, stop=(kx == 2),
                            tile_position=(0, 64 * h),
                        )
                for h in range(2):
                    pslc = ph[h]
                    psv = pslc.rearrange("c h (w two) -> c h w two", two=2)
                    te = tmp.tile([64, 16, 16], F32, tag="te")
                    nc.vector.tensor_copy(te, psv[:, :, :, 0])
                    t1 = tmp.tile([64, 16, 16], F32, tag="t1")
                    nc.vector.tensor_max(t1, psv[:, :, :, 1], te)
                    t1v = t1.rearrange("c (h two) w -> c h two w", two=2)
                    t2 = tmp.tile([64, 8, 16], F32, tag="t2")
                    nc.vector.tensor_max(t2, t1v[:, :, 0, :], t1v[:, :, 1, :])
                    yp0 = 8 * h
                    nc.scalar.sign(
                        AB2[0:64, b, 1 + yp0 : 9 + yp0, 1:17], t2,
                        bias=thc_s[0:64, 1:2],
                    )
                nc.sync.dma_start(
                    out=AB2[64:128, b, 0:17, :], in_=AB2[0:64, b, 1:18, :]
                )

            w5_s = cload(p_w5, [128, 2, 9, 128], FP8, "w5")
            w6_s = cload(p_w6, [128, 2, 2, 9, 128], FP8, "w6")

            # ---- L3 (64->128) ----
            for ci in range(16):  # 2 images per chunk
                b0 = 2 * ci
                ps = psum.tile([128, 2, 16, 16], F32, tag="ps")
                for kx in range(3):
                    nc.tensor.matmul(
                        ps, w3p_s[:, kx, :],
                        AB2[:, b0 : b0 + 2, 0:16, kx : kx + 16],
                        start=(kx == 0), stop=False,
                    )
                for kx in range(3):
                    nc.tensor.matmul(
                        ps, w3s_s[:, kx, :],
                        AB2[:, b0 : b0 + 2, 2:18, kx : kx + 16],
                        start=False, stop=(kx == 2),
                    )
                nc.scalar.sign(A3[:, b0 : b0 + 2, 1:17, 1:17], ps, bias=th(2))

            w7_s = cload(p_w7, [128, 4, 2, 9, 128], FP8, "w7")

            # ---- L4 (128->128) + pool ----
            for ci in range(16):
                b0 = 2 * ci
                ps = psum.tile([128, 2, 16, 16], F32, tag="ps")
                for t, (ky, kx) in enumerate(TAPS):
                    nc.tensor.matmul(
                        ps, w4_s[:, t, :],
                        A3[:, b0 : b0 + 2, ky : ky + 16, kx : kx + 16],
                        start=(t == 0), stop=(t == 8),
                    )
                psv = ps.rearrange("c b h (w two) -> c b h w two", two=2)
                te = tmp.tile([128, 2, 16, 8], F32, tag="te")
                nc.vector.tensor_copy(te, psv[:, :, :, :, 0])
                t1 = tmp.tile([128, 2, 16, 8], F32, tag="t1")
                nc.vector.tensor_max(t1, psv[:, :, :, :, 1], te)
                t1v = t1.rearrange("c b (h two) w -> c b h two w", two=2)
                t2 = tmp.tile([128, 2, 8, 8], F32, tag="t2")
                nc.vector.tensor_max(t2, t1v[:, :, :, 0, :], t1v[:, :, :, 1, :])
                nc.scalar.sign(A4[:, b0 : b0 + 2, 1:9, 1:9], t2, bias=th(3))

            w8_s = cload(p_w8, [128, 4, 4, 9, 128], FP8, "w8")

            # ---- L5 (128->256) ----
            for ci in range(4):  # 8 images per chunk
                b0 = 8 * ci
                for mg in range(2):
                    ps = psum.tile([128, 8, 8, 8], F32, tag="ps")
                    for t, (ky, kx) in enumerate(TAPS):
                        nc.tensor.matmul(
                            ps, w5_s[:, mg, t, :],
                            A4[:, b0 : b0 + 8, ky : ky + 8, kx : kx + 8],
                            start=(t == 0), stop=(t == 8),
                        )
                    nc.scalar.sign(
                        A5[:, mg, b0 : b0 + 8, 1:9, 1:9], ps, bias=th(4 + mg)
                    )

            w1t_s = cload(p_w1t, [128, 16, 1024], FP8, "w1t")
            thfc1_s = consts.tile([B, 1024], F32, tag="thfc1")
            nc.sync.dma_start(out=thfc1_s, in_=p_thfc1[:, :].to_broadcast((B, 1024)))

            # ---- L6 (256->256) + pool ----
            for ci in range(4):
                b0 = 8 * ci
                for mg in range(2):
                    ps = psum.tile([128, 8, 8, 8], F32, tag="ps")
                    i = 0
                    for kg in range(2):
                        for t, (ky, kx) in enumerate(TAPS):
                            nc.tensor.matmul(
                                ps, w6_s[:, mg, kg, t, :],
                                A5[:, kg, b0 : b0 + 8, ky : ky + 8, kx : kx + 8],
                                start=(i == 0), stop=(i == 17),
                            )
                            i += 1
                    psv = ps.rearrange("c b h (w two) -> c b h w two", two=2)
                    te = tmp.tile([128, 8, 8, 4], F32, tag="te")
                    nc.vector.tensor_copy(te, psv[:, :, :, :, 0])
                    t1 = tmp.tile([128, 8, 8, 4], F32, tag="t1")
                    nc.vector.tensor_max(t1, psv[:, :, :, :, 1], te)
                    t1v = t1.rearrange("c b (h two) w -> c b h two w", two=2)
                    t2 = tmp.tile([128, 8, 4, 4], F32, tag="t2")
                    nc.vector.tensor_max(
                        t2, t1v[:, :, :, 0, :], t1v[:, :, :, 1, :]
                    )
                    nc.scalar.sign(
                        A6[:, mg, b0 : b0 + 8, 1:5, 1:5], t2, bias=th(6 + mg)
                    )

            w2t_s = cload(p_w2t, [128, 8, 512], BF16, "w2t")
            w3t_s = cload(p_w3t, [128, 4, 200], BF16, "w3t")
            thfc2_s = consts.tile([B, 512], F32, tag="thfc2")
            nc.sync.dma_start(out=thfc2_s, in_=p_thfc2[:, :].to_broadcast((B, 512)))

            # ---- L7 (256->512) ----
            for mg in range(4):
                ps = psum.tile([128, B, 4, 4], F32, tag="ps")
                i = 0
                for kg in range(2):
                    for t, (ky, kx) in enumerate(TAPS):
                        nc.tensor.matmul(
                            ps, w7_s[:, mg, kg, t, :],
                            A6[:, kg, :, ky : ky + 4, kx : kx + 4],
                            start=(i == 0), stop=(i == 17),
                        )
                        i += 1
                nc.scalar.sign(A7[:, mg, :, 1:5, 1:5], ps, bias=th(8 + mg))

            # ---- L8 (512->512) + pool -> HT ----
            for mg in range(4):
                ps = psum.tile([128, B, 4, 4], F32, tag="ps")
                i = 0
                for kg in range(4):
                    for t, (ky, kx) in enumerate(TAPS):
                        nc.tensor.matmul(
                            ps, w8_s[:, mg, kg, t, :],
                            A7[:, kg, :, ky : ky + 4, kx : kx + 4],
                            start=(i == 0), stop=(i == 35),
                        )
                        i += 1
                psv = ps.rearrange("c b h (w two) -> c b h w two", two=2)
                te = tmp.tile([128, B, 4, 2], F32, tag="te")
                nc.vector.tensor_copy(te, psv[:, :, :, :, 0])
                t1 = tmp.tile([128, B, 4, 2], F32, tag="t1")
                nc.vector.tensor_max(t1, psv[:, :, :, :, 1], te)
                t1v = t1.rearrange("c b (h two) w -> c b h two w", two=2)
                t2 = tmp.tile([128, B, 2, 2], F32, tag="t2")
                nc.vector.tensor_max(t2, t1v[:, :, :, 0, :], t1v[:, :, :, 1, :])
                for s in range(4):
                    nc.scalar.sign(
                        HT[:, 4 * mg + s, :], t2[:, :, s // 2, s % 2],
                        bias=th(12 + mg),
                    )

            # ---- FC1 (2048->1024), +-1 inputs ----
            psa = psum.tile([B, 512], F32, tag="ps")
            psb = psum.tile([B, 512], F32, tag="ps")
            DRM = mybir.MatmulPerfMode.DoubleRow
            for t in range(8):  # DoubleRow: K=256 per pass (fp8)
                nc.tensor.matmul(
                    psa, HT[:, 2 * t : 2 * t + 2, :],
                    w1t_s[:, 2 * t : 2 * t + 2, 0:512],
                    start=(t == 0), stop=(t == 7), perf_mode=DRM,
                )
            for t in range(8):
                nc.tensor.matmul(
                    psb, HT[:, 2 * t : 2 * t + 2, :],
                    w1t_s[:, 2 * t : 2 * t + 2, 512:1024],
                    start=(t == 0), stop=(t == 7), perf_mode=DRM,
                )
            nc.vector.tensor_tensor(
                H2[:, 0:512], psa, thfc1_s[:, 0:512], op=ALU.is_ge
            )
            nc.vector.tensor_tensor(
                H2[:, 512:1024], psb, thfc1_s[:, 512:1024], op=ALU.is_ge
            )
            for j in range(8):
                nc.sync.dma_start(
                    out=H2T[:, j, :], in_=H2[:, 128 * j : 128 * (j + 1)],
                    transpose=True,
                )

            # ---- FC2 (1024->512), {0,1} inputs ----
            ps2 = psum.tile([B, 512], F32, tag="ps")
            for j in range(8):
                nc.tensor.matmul(
                    ps2, H2T[:, j, :], w2t_s[:, j, :],
                    start=(j == 0), stop=(j == 7),
                )
            nc.vector.tensor_tensor(H3, ps2, thfc2_s, op=ALU.is_ge)
            for j in range(4):
                nc.sync.dma_start(
                    out=H3T[:, j, :], in_=H3[:, 128 * j : 128 * (j + 1)],
                    transpose=True,
                )

            # ---- FC3 (512->200), raw accumulator out ----
            ps3 = psum.tile([B, 200], F32, tag="ps")
            for j in range(4):
                nc.tensor.matmul(
                    ps3, H3T[:, j, :], w3t_s[:, j, :],
                    start=(j == 0), stop=(j == 3),
                )
            nc.scalar.copy(OUTS, ps3)
            nc.sync.dma_start(out=p_out[:, :], in_=OUTS)

    nc.finalize()
    return nc


_PROGRAM = None
TRACE = False  # set True (e.g. from test.py) to capture an NTFF profile
LAST_RESULTS = None


def _get_program():
    global _PROGRAM
    if _PROGRAM is None:
        _PROGRAM = build_program()
    return _PROGRAM


# --------------------------------------------------------------------------
# entry point
# --------------------------------------------------------------------------

def kernel(x, conv_ws, bn2d, fc_ws, bn1d, tn):
    I, shared, rowsum3 = _host_prep(x, conv_ws, bn2d, fc_ws, bn1d)

    nc = _get_program()
    in_maps = []
    for c in range(N_CORES):
        m = dict(shared)
        m["im2col"] = np.ascontiguousarray(I[:, B * c : B * (c + 1)])
        in_maps.append(m)

    global LAST_RESULTS
    res = run_bass_kernel_spmd(nc, in_maps, list(range(N_CORES)), trace=TRACE)
    LAST_RESULTS = res
    fc3 = np.concatenate(
        [res.results[c]["out"] for c in range(N_CORES)], axis=0
    ).astype(F32N)  # [256, 200], FC3 sums over {0,1} inputs

    # undo the {0,1} encoding, then TensorNorm (all f32, matching reference)
    true3 = F32N(2.0) * fc3 - rowsum3[None, :]
    tw, tb, tm, tv = [F32N(np.asarray(a)) for a in tn]
    out = (true3 - tm) / np.sqrt(tv + F32N(EPS)) * tw + tb
    return out.astype(F32N)


# revision 22
# speedup vs baseline: 2.0013x; 1.0818x over previous
"""Trainium2 Bass kernel for the CNV binarized CNN (nn_CNV_65549790871624).

Data-parallel over 8 NeuronCores: batch 256 -> 32 images/core. The whole
network runs out of SBUF per core:

  - Input is pre-quantized (8-bit grid, exact in bf16) and im2col'd on host
    so conv1 (C_in=3) is a single K=27 matmul pass.
  - Binary convs run as 3x3 tap-accumulation matmuls into PSUM. Activations
    are +-1 in fp8 stored with a one-pixel zero halo, so every tap is a pure
    strided SBUF read. For the C_in=64 layers (conv2/conv3) a row-shifted
    copy of the activation lives in partitions 64..127, letting vertically
    adjacent tap pairs fuse into single K=128 matmuls (6 passes instead
    of 9).
  - BatchNorm+sign folds to a per-channel threshold compare, executed as one
    ScalarE activation (Sign(x - t)) straight out of PSUM. All conv sums are
    exact integers in f32, so results bit-match the f32 reference up to an
    ~1e-7-wide decision boundary.
  - MaxPool runs on VectorE over the raw conv integers (pooling commutes
    with the monotone threshold), halving ScalarE work.
  - FC layers: FC1 consumes the +-1 activations with a K-permuted weight
    layout chosen so the conv drain writes lhsT tiles directly (no
    transpose). FC2/FC3 consume {0,1} activations (the +-1 correction folds
    into thresholds / a host-side affine), with bf16 DMA-transposes between
    layers. The final TensorNorm affine runs on host in f32, bit-matching
    the reference.
"""

import sys

if "/opt/trn_rl_repo" not in sys.path:
    sys.path.insert(0, "/opt/trn_rl_repo")

from itertools import product

import numpy as np

import concourse.bass as bass
import concourse.mybir as mybir
import concourse.tile as tile
from concourse import bacc
from concourse.bass_utils import run_bass_kernel_spmd

F32N = np.float32
N_CORES = 8
B = 32  # images per core
EPS = 1e-4

FP8 = mybir.dt.float8e4
BF16 = mybir.dt.bfloat16
F32 = mybir.dt.float32
FP8N = mybir.dt.np(FP8)
BF16N = mybir.dt.np(BF16)

TAPS = list(product(range(3), range(3)))  # (ky, kx), tap index t = ky*3+kx
ALU = mybir.AluOpType


# --------------------------------------------------------------------------
# host-side preprocessing
# --------------------------------------------------------------------------

def _thr(bn):
    """Fold BN+sign into a threshold: sign((x-m)*inv+b) == (x >= t), inv>0."""
    g, b, m, v = [np.asarray(a, F32N) for a in bn]
    inv32 = (g / np.sqrt(v + F32N(EPS))).astype(F32N)
    t = m.astype(np.float64) - b.astype(np.float64) / inv32.astype(np.float64)
    return t.astype(F32N)


def _binarize_w(w):
    return np.where(np.asarray(w, F32N) >= 0, F32N(1.0), F32N(-1.0))


def _host_prep(x, conv_ws, bn2d, fc_ws, bn1d):
    x = np.asarray(x, F32N)
    # QuantIdentity: 8-bit, scale 2^-7, bit-matching the jax reference ops
    h = F32N(2.0) * x - F32N(1.0)
    r = np.clip(np.round(h * F32N(128.0)), -128.0, 127.0).astype(F32N)
    q = (r * F32N(2.0**-7)).astype(F32N)  # [256, 3, 32, 32]

    wb = [_binarize_w(w) for w in conv_ws]
    th = [_thr(t) for t in bn2d]

    # L1 im2col: I[3t+c, b, yo, xo] = qpad[c, b, yo+ky, xo+kx]
    qp = np.zeros((3, 256, 34, 34), F32N)
    qp[:, :, 1:33, 1:33] = q.transpose(1, 0, 2, 3)
    I = np.zeros((27, 256, 32, 32), F32N)
    for t, (ky, kx) in enumerate(TAPS):
        I[3 * t : 3 * t + 3] = qp[:, :, ky : ky + 32, kx : kx + 32]
    I = I.astype(BF16N)

    # K is padded to the full 128 partitions everywhere (zero weight rows):
    # sub-128-K matmuls keep the PE HAM-throttled at 1.2 GHz for their whole
    # phase (measured), and zero rows make the extra partitions harmless.
    w1 = np.zeros((27, 64), F32N)
    for t, (ky, kx) in enumerate(TAPS):
        w1[3 * t : 3 * t + 3, :] = wb[0][:, :, ky, kx].T

    def pack_pair_single(w):  # [O, 64, 3, 3] -> wp [128,3,O], ws [128,3,O]
        O = w.shape[0]
        wp = np.zeros((128, 3, O), F32N)
        ws = np.zeros((128, 3, O), F32N)  # rows 64:128 stay zero
        for kx in range(3):
            wp[0:64, kx, :] = w[:, :, 0, kx].T
            wp[64:128, kx, :] = w[:, :, 1, kx].T
            ws[0:64, kx, :] = w[:, :, 2, kx].T
        return wp, ws

    w2p, w2s = pack_pair_single(wb[1])
    w3p, w3s = pack_pair_single(wb[2])

    def pack_full(w):  # [O, C, 3, 3] -> [128, MG, KG, 9, 128]
        O, C = w.shape[0], w.shape[1]
        MG, KG = O // 128, C // 128
        out = np.zeros((128, MG, KG, 9, 128), F32N)
        for mg, kg in product(range(MG), range(KG)):
            for t, (ky, kx) in enumerate(TAPS):
                out[:, mg, kg, t, :] = w[
                    128 * mg : 128 * (mg + 1), 128 * kg : 128 * (kg + 1), ky, kx
                ].T
        return out

    w4 = pack_full(wb[3]).reshape(128, 9, 128)
    w5 = pack_full(wb[4]).reshape(128, 2, 9, 128)
    w6 = pack_full(wb[5])  # [128,2,2,9,128]
    w7 = pack_full(wb[6])  # [128,4,2,9,128]
    w8 = pack_full(wb[7])  # [128,4,4,9,128]

    # negated thresholds packed [128, 16]: L1,L2,L3,L4,L5g0,L5g1,L6g0,L6g1,
    # L7g0..3, L8g0..3
    thc = np.zeros((128, 16), F32N)
    # cols 0/1 replicated into partitions 64:128 for col-tiled psum halves
    thc[0:64, 0] = -th[0]
    thc[64:128, 0] = -th[0]
    thc[0:64, 1] = -th[1]
    thc[64:128, 1] = -th[1]
    thc[:, 2] = -th[2]
    thc[:, 3] = -th[3]
    for mg in range(2):
        thc[:, 4 + mg] = -th[4][128 * mg : 128 * (mg + 1)]
        thc[:, 6 + mg] = -th[5][128 * mg : 128 * (mg + 1)]
    for mg in range(4):
        thc[:, 8 + mg] = -th[6][128 * mg : 128 * (mg + 1)]
        thc[:, 12 + mg] = -th[7][128 * mg : 128 * (mg + 1)]

    # FC prep
    w1b, w2b, w3b = [_binarize_w(w) for w in fc_ws]
    w1t = np.zeros((128, 16, 1024), F32N)
    for g, j in product(range(4), range(4)):
        ks = (128 * g + np.arange(128)) * 4 + j
        w1t[:, 4 * g + j, :] = w1b[:, ks].T
    th_fc1 = _thr(bn1d[0]).reshape(1, 1024)
    t2d = _thr(bn1d[1]).astype(np.float64)
    th_fc2 = ((t2d + w2b.sum(axis=1, dtype=np.float64)) / 2.0).astype(F32N)
    th_fc2 = th_fc2.reshape(1, 512)
    w2t = np.zeros((128, 8, 512), F32N)
    for j in range(8):
        w2t[:, j, :] = w2b[:, 128 * j : 128 * (j + 1)].T
    w3t = np.zeros((128, 4, 200), F32N)
    for j in range(4):
        w3t[:, j, :] = w3b[:, 128 * j : 128 * (j + 1)].T
    rowsum3 = w3b.sum(axis=1, dtype=np.float64).astype(F32N)

    shared = {
        "w1": w1.astype(BF16N),
        "w2p": w2p.astype(FP8N), "w2s": w2s.astype(FP8N),
        "w3p": w3p.astype(FP8N), "w3s": w3s.astype(FP8N),
        "w4": w4.astype(FP8N), "w5": w5.astype(FP8N), "w6": w6.astype(FP8N),
        "w7": w7.astype(FP8N), "w8": w8.astype(FP8N),
        "thc": thc,
        "w1t": w1t.astype(FP8N),
        "w2t": w2t.astype(BF16N), "w3t": w3t.astype(BF16N),
        "thfc1": th_fc1, "thfc2": th_fc2,
    }
    return I, shared, rowsum3


# --------------------------------------------------------------------------
# bass program
# --------------------------------------------------------------------------

def build_program(cache_buster: str | None = None):
    nc = bacc.Bacc("TRN2", target_bir_lowering=False, num_devices=N_CORES)
    if cache_buster:
        # unused dram scratch whose name varies: busts the jax/NEFF compile
        # cache when we want to A/B walrus flags on an identical program
        nc.dram_tensor(f"cachebust_{cache_buster}", [1, 1], F32)

    def par(name, shape, dt):
        return nc.declare_dram_parameter(name, shape, dt, isOutput=False)

    p_im = par("im2col", [27, B, 32, 32], BF16)
    p_w1 = par("w1", [27, 64], BF16)
    p_w2p = par("w2p", [128, 3, 64], FP8)
    p_w2s = par("w2s", [128, 3, 64], FP8)
    p_w3p = par("w3p", [128, 3, 128], FP8)
    p_w3s = par("w3s", [128, 3, 128], FP8)
    p_w4 = par("w4", [128, 9, 128], FP8)
    p_w5 = par("w5", [128, 2, 9, 128], FP8)
    p_w6 = par("w6", [128, 2, 2, 9, 128], FP8)
    p_w7 = par("w7", [128, 4, 2, 9, 128], FP8)
    p_w8 = par("w8", [128, 4, 4, 9, 128], FP8)
    p_thc = par("thc", [128, 16], F32)
    p_w1t = par("w1t", [128, 16, 1024], FP8)
    p_w2t = par("w2t", [128, 8, 512], BF16)
    p_w3t = par("w3t", [128, 4, 200], BF16)
    p_thfc1 = par("thfc1", [1, 1024], F32)
    p_thfc2 = par("thfc2", [1, 512], F32)
    p_out = nc.declare_dram_parameter("out", [B, 200], F32, isOutput=True)

    with tile.TileContext(nc) as tc:
        with (
            tc.tile_pool(name="consts", bufs=1) as consts,
            tc.tile_pool(name="acts", bufs=1) as acts,
            tc.tile_pool(name="imp", bufs=2) as imp,
            tc.tile_pool(name="tmp", bufs=4) as tmp,
            tc.tile_pool(name="psum", bufs=8, space="PSUM") as psum,
        ):
            # ---- load constants ----
            def cload(p, shape, dt, tag):
                t = consts.tile(shape, dt, tag=tag)
                nc.sync.dma_start(out=t, in_=p[(slice(None),) * len(shape)])
                return t

            # only what L1 needs up front; heavier weight loads are emitted
            # between layers so they queue behind each layer's small DMAs and
            # stream in during compute instead of blocking the first im2col
            thc_s = cload(p_thc, [128, 16], F32, "thc")
            w1_s = cload(p_w1, [27, 64], BF16, "w1")
            w2p_s = cload(p_w2p, [128, 3, 64], FP8, "w2p")
            w2s_s = cload(p_w2s, [128, 3, 64], FP8, "w2s")

            def th(col, parts=128):
                return thc_s[0:parts, col : col + 1]

            # ---- activation buffers (one-pixel zero halo) ----
            AB1 = acts.tile([128, B, 34, 34], FP8, tag="AB1")
            AB2 = acts.tile([128, B, 18, 18], FP8, tag="AB2")
            A3 = acts.tile([128, B, 18, 18], FP8, tag="A3")
            A4 = acts.tile([128, B, 10, 10], FP8, tag="A4")
            A5 = acts.tile([128, 2, B, 10, 10], FP8, tag="A5")
            A6 = acts.tile([128, 2, B, 6, 6], FP8, tag="A6")
            A7 = acts.tile([128, 4, B, 6, 6], FP8, tag="A7")
            HT = acts.tile([128, 16, B], FP8, tag="HT")
            H2 = acts.tile([B, 1024], BF16, tag="H2")
            H2T = acts.tile([128, 8, B], BF16, tag="H2T")
            H3 = acts.tile([B, 512], BF16, tag="H3")
            H3T = acts.tile([128, 4, B], BF16, tag="H3T")
            OUTS = acts.tile([B, 200], F32, tag="OUTS")

            def zero_halo2(buf, parts, S):  # buf [128, B, S+2, S+2]
                nc.vector.memset(buf[0:parts, :, 0, :], 0.0)
                nc.vector.memset(buf[0:parts, :, S + 1, :], 0.0)
                nc.gpsimd.memset(buf[0:parts, :, :, 0], 0.0)
                nc.gpsimd.memset(buf[0:parts, :, :, S + 1], 0.0)

            def zero_halo3(buf, S):  # buf [128, G, B, S+2, S+2]
                nc.vector.memset(buf[:, :, :, 0, :], 0.0)
                nc.vector.memset(buf[:, :, :, S + 1, :], 0.0)
                nc.gpsimd.memset(buf[:, :, :, :, 0], 0.0)
                nc.gpsimd.memset(buf[:, :, :, :, S + 1], 0.0)

            zero_halo2(AB1, 64, 32)
            zero_halo2(AB2, 64, 16)
            # B-half bottom rows are never written by the shift-copy but are
            # read (x0-weighted) by the K-padded "single" matmuls
            nc.vector.memset(AB1[64:128, :, 33, :], 0.0)
            nc.vector.memset(AB2[64:128, :, 17, :], 0.0)
            zero_halo2(A3, 128, 16)
            zero_halo2(A4, 128, 8)
            zero_halo3(A5, 8)
            zero_halo3(A6, 4)
            zero_halo3(A7, 4)

            # ---- L1: im2col matmul, K=27 padded to 128 ----
            # L1 emitter: chunk DMA every 4th image, 2 col-tiled matmuls,
            # 2 sign drains, B-half shift copy
            im_cur = {}

            def emit_l1(b):
                if b % 4 == 0:
                    im_new = imp.tile([27, 4, 32, 32], BF16, tag="imc")
                    nc.sync.dma_start(out=im_new, in_=p_im[:, b : b + 4, :, :])
                    im_cur["t"] = im_new
                im_s = im_cur["t"]
                bi = b % 4
                ps = psum.tile([128, 16, 32], F32, tag="ps")
                nc.tensor.matmul(
                    ps[0:64], w1_s, im_s[:, bi, 0:16, :],
                    start=True, stop=True, tile_position=(0, 0),
                )
                nc.tensor.matmul(
                    ps[64:128], w1_s, im_s[:, bi, 16:32, :],
                    start=True, stop=True, tile_position=(0, 64),
                )
                nc.scalar.sign(
                    AB1[0:64, b, 1:17, 1:33], ps[0:64], bias=thc_s[0:64, 0:1]
                )
                nc.scalar.sign(
                    AB1[0:64, b, 17:33, 1:33], ps[64:128],
                    bias=thc_s[64:128, 0:1],
                )
                nc.sync.dma_start(
                    out=AB1[64:128, b, 0:33, :], in_=AB1[0:64, b, 1:34, :]
                )

            w3p_s = cload(p_w3p, [128, 3, 128], FP8, "w3p")
            w3s_s = cload(p_w3s, [128, 3, 128], FP8, "w3s")
            w4_s = cload(p_w4, [128, 9, 128], FP8, "w4")

            # ---- L2 (64->64) + pool, 2-way col tiling over y-halves.
            # Each half gets its own PSUM bank: start=True clears has_written
            # for the whole bank, so interleaved accumulation of both halves
            # must not share one.
            def emit_l2(b):
                psA = psum.tile([128, 16, 32], F32, tag="ps")
                psB = psum.tile([128, 16, 32], F32, tag="ps")
                ph = [psA[0:64], psB[64:128]]
                y0s = [0, 16]
                for kx in range(3):  # ky=0,1 pairs (K=128)
                    for h in range(2):
                        nc.tensor.matmul(
                            ph[h], w2p_s[:, kx, :],
                            AB1[:, b, y0s[h] : y0s[h] + 16, kx : kx + 32],
                            start=(kx == 0), stop=False,
                            tile_position=(0, 64 * h),
                        )
                for kx in range(3):  # ky=2 singles (K padded to 128)
                    for h in range(2):
                        nc.tensor.matmul(
                            ph[h], w2s_s[:, kx, :],
                            AB1[:, b, y0s[h] + 2 : y0s[h] + 18, kx : kx + 32],
                            start=False, stop=(kx == 2),
                            tile_position=(0, 64 * h),
                        )
                for h in range(2):
                    pslc = ph[h]
                    psv = pslc.rearrange("c h (w two) -> c h w two", two=2)
                    te = tmp.tile([64, 16, 16], F32, tag="te")
                    nc.vector.tensor_copy(te, psv[:, :, :, 0])
                    t1 = tmp.tile([64, 16, 16], F32, tag="t1")
                    nc.vector.tensor_max(t1, psv[:, :, :, 1], te)
                    t1v = t1.rearrange("c (h two) w -> c h two w", two=2)
                    t2 = tmp.tile([64, 8, 16], F32, tag="t2")
                    nc.vector.tensor_max(t2, t1v[:, :, 0, :], t1v[:, :, 1, :])
                    yp0 = 8 * h
                    nc.scalar.sign(
                        AB2[0:64, b, 1 + yp0 : 9 + yp0, 1:17], t2,
                        bias=thc_s[0:64, 1:2],
                    )
                nc.sync.dma_start(
                    out=AB2[64:128, b, 0:17, :], in_=AB2[0:64, b, 1:18, :]
                )

            # interleave: L1 leads L2 by 4 images so L2's dense K=128
            # matmuls keep the in-order PE stream busy while L1's ACT
            # drains recycle psum slots (also warms HAM early)
            for b in range(4):
                emit_l1(b)
            for b in range(B):
                if b + 4 < B:
                    emit_l1(b + 4)
                emit_l2(b)

            w5_s = cload(p_w5, [128, 2, 9, 128], FP8, "w5")
            w6_s = cload(p_w6, [128, 2, 2, 9, 128], FP8, "w6")

            # ---- L3 (64->128) ----
            for ci in range(16):  # 2 images per chunk
                b0 = 2 * ci
                ps = psum.tile([128, 2, 16, 16], F32, tag="ps")
                for kx in range(3):
                    nc.tensor.matmul(
                        ps, w3p_s[:, kx, :],
                        AB2[:, b0 : b0 + 2, 0:16, kx : kx + 16],
                        start=(kx == 0), stop=False,
                    )
                for kx in range(3):
                    nc.tensor.matmul(
                        ps, w3s_s[:, kx, :],
                        AB2[:, b0 : b0 + 2, 2:18, kx : kx + 16],
                        start=False, stop=(kx == 2),
                    )
                nc.scalar.sign(A3[:, b0 : b0 + 2, 1:17, 1:17], ps, bias=th(2))

            w7_s = cload(p_w7, [128, 4, 2, 9, 128], FP8, "w7")

            # ---- L4 (128->128) + pool ----
            for ci in range(16):
                b0 = 2 * ci
                ps = psum.tile([128, 2, 16, 16], F32, tag="ps")
                for t, (ky, kx) in enumerate(TAPS):
                    nc.tensor.matmul(
                        ps, w4_s[:, t, :],
                        A3[:, b0 : b0 + 2, ky : ky + 16, kx : kx + 16],
                        start=(t == 0), stop=(t == 8),
                    )
                psv = ps.rearrange("c b h (w two) -> c b h w two", two=2)
                te = tmp.tile([128, 2, 16, 8], F32, tag="te")
                nc.vector.tensor_copy(te, psv[:, :, :, :, 0])
                t1 = tmp.tile([128, 2, 16, 8], F32, tag="t1")
                nc.vector.tensor_max(t1, psv[:, :, :, :, 1], te)
                t1v = t1.rearrange("c b (h two) w -> c b h two w", two=2)
                t2 = tmp.tile([128, 2, 8, 8], F32, tag="t2")
                nc.vector.tensor_max(t2, t1v[:, :, :, 0, :], t1v[:, :, :, 1, :])
                nc.scalar.sign(A4[:, b0 : b0 + 2, 1:9, 1:9], t2, bias=th(3))

            w8_s = cload(p_w8, [128, 4, 4, 9, 128], FP8, "w8")

            # ---- L5 (128->256) ----
            for ci in range(4):  # 8 images per chunk
                b0 = 8 * ci
                for mg in range(2):
                    ps = psum.tile([128, 8, 8, 8], F32, tag="ps")
                    for t, (ky, kx) in enumerate(TAPS):
                        nc.tensor.matmul(
                            ps, w5_s[:, mg, t, :],
                            A4[:, b0 : b0 + 8, ky : ky + 8, kx : kx + 8],
                            start=(t == 0), stop=(t == 8),
                        )
                    nc.scalar.sign(
                        A5[:, mg, b0 : b0 + 8, 1:9, 1:9], ps, bias=th(4 + mg)
                    )

            w1t_s = cload(p_w1t, [128, 16, 1024], FP8, "w1t")
            thfc1_s = consts.tile([B, 1024], F32, tag="thfc1")
            nc.sync.dma_start(out=thfc1_s, in_=p_thfc1[:, :].to_broadcast((B, 1024)))

            # ---- L6 (256->256) + pool ----
            for ci in range(4):
                b0 = 8 * ci
                for mg in range(2):
                    ps = psum.tile([128, 8, 8, 8], F32, tag="ps")
                    i = 0
                    for kg in range(2):
                        for t, (ky, kx) in enumerate(TAPS):
                            nc.tensor.matmul(
                                ps, w6_s[:, mg, kg, t, :],
                                A5[:, kg, b0 : b0 + 8, ky : ky + 8, kx : kx + 8],
                                start=(i == 0), stop=(i == 17),
                            )
                            i += 1
                    psv = ps.rearrange("c b h (w two) -> c b h w two", two=2)
                    te = tmp.tile([128, 8, 8, 4], F32, tag="te")
                    nc.vector.tensor_copy(te, psv[:, :, :, :, 0])
                    t1 = tmp.tile([128, 8, 8, 4], F32, tag="t1")
                    nc.vector.tensor_max(t1, psv[:, :, :, :, 1], te)
                    t1v = t1.rearrange("c b (h two) w -> c b h two w", two=2)
                    t2 = tmp.tile([128, 8, 4, 4], F32, tag="t2")
                    nc.vector.tensor_max(
                        t2, t1v[:, :, :, 0, :], t1v[:, :, :, 1, :]
                    )
                    nc.scalar.sign(
                        A6[:, mg, b0 : b0 + 8, 1:5, 1:5], t2, bias=th(6 + mg)
                    )

            w2t_s = cload(p_w2t, [128, 8, 512], BF16, "w2t")
            w3t_s = cload(p_w3t, [128, 4, 200], BF16, "w3t")
            thfc2_s = consts.tile([B, 512], F32, tag="thfc2")
            nc.sync.dma_start(out=thfc2_s, in_=p_thfc2[:, :].to_broadcast((B, 512)))

            # ---- L7 (256->512) ----
            for mg in range(4):
                ps = psum.tile([128, B, 4, 4], F32, tag="ps")
                i = 0
                for kg in range(2):
                    for t, (ky, kx) in enumerate(TAPS):
                        nc.tensor.matmul(
                            ps, w7_s[:, mg, kg, t, :],
                            A6[:, kg, :, ky : ky + 4, kx : kx + 4],
                            start=(i == 0), stop=(i == 17),
                        )
                        i += 1
                nc.scalar.sign(A7[:, mg, :, 1:5, 1:5], ps, bias=th(8 + mg))

            # ---- L8 (512->512) + pool -> HT ----
            for mg in range(4):
                ps = psum.tile([128, B, 4, 4], F32, tag="ps")
                i = 0
                for kg in range(4):
                    for t, (ky, kx) in enumerate(TAPS):
                        nc.tensor.matmul(
                            ps, w8_s[:, mg, kg, t, :],
                            A7[:, kg, :, ky : ky + 4, kx : kx + 4],
                            start=(i == 0), stop=(i == 35),
                        )
                        i += 1
                psv = ps.rearrange("c b h (w two) -> c b h w two", two=2)
                te = tmp.tile([128, B, 4, 2], F32, tag="te")
                nc.vector.tensor_copy(te, psv[:, :, :, :, 0])
                t1 = tmp.tile([128, B, 4, 2], F32, tag="t1")
                nc.vector.tensor_max(t1, psv[:, :, :, :, 1], te)
                t1v = t1.rearrange("c b (h two) w -> c b h two w", two=2)
                t2 = tmp.tile([128, B, 2, 2], F32, tag="t2")
                nc.vector.tensor_max(t2, t1v[:, :, :, 0, :], t1v[:, :, :, 1, :])
                for s in range(4):
                    nc.scalar.sign(
                        HT[:, 4 * mg + s, :], t2[:, :, s // 2, s % 2],
                        bias=th(12 + mg),
                    )

            # ---- FC1 (2048->1024), +-1 inputs ----
            psa = psum.tile([B, 512], F32, tag="ps")
            psb = psum.tile([B, 512], F32, tag="ps")
            DRM = mybir.MatmulPerfMode.DoubleRow
            for t in range(8):  # DoubleRow: K=256 per pass (fp8)
                nc.tensor.matmul(
                    psa, HT[:, 2 * t : 2 * t + 2, :],
                    w1t_s[:, 2 * t : 2 * t + 2, 0:512],
                    start=(t == 0), stop=(t == 7), perf_mode=DRM,
                )
            for t in range(8):
                nc.tensor.matmul(
                    psb, HT[:, 2 * t : 2 * t + 2, :],
                    w1t_s[:, 2 * t : 2 * t + 2, 512:1024],
                    start=(t == 0), stop=(t == 7), perf_mode=DRM,
                )
            nc.vector.tensor_tensor(
                H2[:, 0:512], psa, thfc1_s[:, 0:512], op=ALU.is_ge
            )
            nc.vector.tensor_tensor(
                H2[:, 512:1024], psb, thfc1_s[:, 512:1024], op=ALU.is_ge
            )
            for j in range(8):
                nc.sync.dma_start(
                    out=H2T[:, j, :], in_=H2[:, 128 * j : 128 * (j + 1)],
                    transpose=True,
                )

            # ---- FC2 (1024->512), {0,1} inputs ----
            ps2 = psum.tile([B, 512], F32, tag="ps")
            for j in range(8):
                nc.tensor.matmul(
                    ps2, H2T[:, j, :], w2t_s[:, j, :],
                    start=(j == 0), stop=(j == 7),
                )
            nc.vector.tensor_tensor(H3, ps2, thfc2_s, op=ALU.is_ge)
            for j in range(4):
                nc.sync.dma_start(
                    out=H3T[:, j, :], in_=H3[:, 128 * j : 128 * (j + 1)],
                    transpose=True,
                )

            # ---- FC3 (512->200), raw accumulator out ----
            ps3 = psum.tile([B, 200], F32, tag="ps")
            for j in range(4):
                nc.tensor.matmul(
                    ps3, H3T[:, j, :], w3t_s[:, j, :],
                    start=(j == 0), stop=(j == 3),
                )
            nc.scalar.copy(OUTS, ps3)
            nc.sync.dma_start(out=p_out[:, :], in_=OUTS)

    nc.finalize()
    return nc


_PROGRAM = None
TRACE = False  # set True (e.g. from test.py) to capture an NTFF profile
LAST_RESULTS = None


def _get_program():
    global _PROGRAM
    if _PROGRAM is None:
        _PROGRAM = build_program()
    return _PROGRAM


# --------------------------------------------------------------------------
# entry point
# --------------------------------------------------------------------------

def kernel(x, conv_ws, bn2d, fc_ws, bn1d, tn):
    I, shared, rowsum3 = _host_prep(x, conv_ws, bn2d, fc_ws, bn1d)

    nc = _get_program()
    in_maps = []
    for c in range(N_CORES):
        m = dict(shared)
        m["im2col"] = np.ascontiguousarray(I[:, B * c : B * (c + 1)])
        in_maps.append(m)

    global LAST_RESULTS
    res = run_bass_kernel_spmd(nc, in_maps, list(range(N_CORES)), trace=TRACE)
    LAST_RESULTS = res
    fc3 = np.concatenate(
        [res.results[c]["out"] for c in range(N_CORES)], axis=0
    ).astype(F32N)  # [256, 200], FC3 sums over {0,1} inputs

    # undo the {0,1} encoding, then TensorNorm (all f32, matching reference)
    true3 = F32N(2.0) * fc3 - rowsum3[None, :]
    tw, tb, tm, tv = [F32N(np.asarray(a)) for a in tn]
    out = (true3 - tm) / np.sqrt(tv + F32N(EPS)) * tw + tb
    return out.astype(F32N)
